# revision 21
# baseline (speedup 1.0000x reference)
"""Multi-head attention (Whisper-style, with additive mask) on 8 trn2 NeuronCores.

Batch-parallel: core b computes batch element b entirely on-device:
  q/k/v projections, scores (+mask) -> qk output, softmax, PV, out projection.

Host does layout only: batch slicing, weight transposes (np.ascontiguousarray(W.T)),
mask transpose, and a structural classification of the mask into 128x128 blocks
(all-zero / all--inf / general) that specializes the generated program. All
arithmetic (projections, scores, mask addition, softmax, output projection)
happens on the NeuronCores.
"""
import sys
import os
import functools

sys.path.insert(0, "/opt/trn_rl_repo")
os.environ.setdefault("MYCRO_LOCAL_CACHE", "1")

import numpy as np

B, S, D, H = 8, 1024, 1024, 16
DH = D // H          # 64
P = 128              # partitions
NB = S // P          # 8 blocks of 128
NCORES = 8
SCALE2 = 0.125       # dh ** -0.5 (exact power of two; applied to q side)

_f32 = None  # set lazily


def _dts():
    import concourse.mybir as mybir
    return mybir.dt.float32, mybir.dt.float32r, mybir.dt.bfloat16


def _classify_mask(mask):
    """Classify 128x128 blocks of mask: 0=all-zero, 1=all--inf, 2=general.

    Returns a hashable structure key driving program generation.
    """
    cls = np.zeros((NB, NB), np.int8)
    for i in range(NB):
        for j in range(NB):
            blk = mask[i * P:(i + 1) * P, j * P:(j + 1) * P]
            if np.all(blk == 0.0):
                cls[i][j] = 0
            elif np.all(np.isneginf(blk)):
                cls[i][j] = 1
            else:
                cls[i][j] = 2

    # pass1 (rows = q blocks): trailing all--inf run handled by preset staging
    fin_end = []
    p1_cls = []  # per i: tuple of classes for j < fin blocks (interior -inf -> general)
    for i in range(NB):
        nz = [j for j in range(NB) if cls[i][j] != 1]
        jmax = max(nz) if nz else -1
        fin_end.append(P * (jmax + 1))
        row = []
        for j in range(jmax + 1):
            c = cls[i][j]
            row.append(2 if c == 1 else int(c))  # interior -inf treated as general
        p1_cls.append(tuple(row))

    # pass2 (rows = k blocks of mask^T): clsT[t][j] = cls[j][t]
    qs_l, qe_l, p2_cls = [], [], []
    for t in range(NB):
        nz = [j for j in range(NB) if cls[j][t] != 1]
        if nz:
            jmin, jmax = min(nz), max(nz)
            qs_l.append(P * jmin)
            qe_l.append(P * (jmax + 1))
            row = []
            for j in range(jmin, jmax + 1):
                c = cls[j][t]
                row.append(2 if c == 1 else int(c))
            p2_cls.append(tuple(row))
        else:
            qs_l.append(0)
            qe_l.append(0)
            p2_cls.append(tuple())

    return (tuple(fin_end), tuple(p1_cls), tuple(qs_l), tuple(qe_l),
            tuple(p2_cls))


@functools.lru_cache(maxsize=8)
def _build(key):
    skip = set(os.environ.get("MHA_SKIP", "").split(","))
    import concourse.bass as bass
    from concourse import bacc
    import concourse.mybir as mybir
    import concourse.tile as tile
    from concourse.masks import make_identity

    f32, f32r, bf16 = _dts()
    Ident = mybir.ActivationFunctionType.Identity
    Exp = mybir.ActivationFunctionType.Exp

    fin_end, p1_cls, qs_l, qe_l, p2_cls = key[:5]

    # pass1 general-block slots (i, j) -> slot index in mask_g
    g1 = []
    for i in range(NB):
        for j, c in enumerate(p1_cls[i]):
            if c == 2:
                g1.append((i, j))
    g1_slot = {ij: n for n, ij in enumerate(g1)}
    # pass2 general-block slots (t, j) -> slot in emask_gT
    g2 = []
    for t in range(NB):
        j0 = qs_l[t] // P
        for jj, c in enumerate(p2_cls[t]):
            if c == 2:
                g2.append((t, j0 + jj))
    g2_slot = {tj: n for n, tj in enumerate(g2)}

    nc = bacc.Bacc(None, target_bir_lowering=False)

    x_h = nc.declare_dram_parameter("x", [S, D], f32, isOutput=False)
    wqT_h = nc.declare_dram_parameter("WqT", [D, D], f32, isOutput=False)
    wkT_h = nc.declare_dram_parameter("WkT", [D, D], f32, isOutput=False)
    wvT_h = nc.declare_dram_parameter("WvT", [D, D], f32, isOutput=False)
    woT_h = nc.declare_dram_parameter("WoT", [D, D], f32, isOutput=False)
    bq_h = nc.declare_dram_parameter("bq", [D], f32, isOutput=False)
    bv_h_d = nc.declare_dram_parameter("bv", [D], f32, isOutput=False)
    bo_h = nc.declare_dram_parameter("bo", [D], f32, isOutput=False)
    mask_h = nc.declare_dram_parameter("mask", [S, S], f32, isOutput=False)
    maskT_h = nc.declare_dram_parameter("maskT", [S, S], f32, isOutput=False)
    out_h = nc.declare_dram_parameter("out", [S, D], f32, isOutput=True)
    qk_h = nc.declare_dram_parameter("qk", [H, S, S], f32, isOutput=True)

    with tile.TileContext(nc) as tc:
        with tc.tile_pool(name="persist", bufs=1) as Pp:
            ident = Pp.tile([P, P], f32, tag="ident")
            make_identity(nc, ident)
            # biases
            bq_raw = Pp.tile([P, NB], f32, tag="bq_raw")
            nc.sync.dma_start(out=bq_raw, in_=bq_h[:].rearrange("(b p) -> p b", p=P))
            bq_s = Pp.tile([P, NB], f32, tag="bq_s")
            nc.scalar.activation(bq_s, bq_raw, mybir.ActivationFunctionType.Copy,
                                 scale=SCALE2)
            bv_hd = Pp.tile([DH, H], f32, tag="bv_hd")
            nc.sync.dma_start(out=bv_hd, in_=bv_h_d[:].rearrange("(h j) -> j h", j=DH))

            qT = Pp.tile([P, NB, S], f32r, tag="qT")
            kT = Pp.tile([P, NB, S], f32r, tag="kT")
            v_pad = Pp.tile([P, NB, H * (DH + 1)], f32r, tag="v_pad")
            # ones columns for the softmax denominators (f32 memset + ACT copy
            # so the f32r tile has a rounding producer)
            ones_src = Pp.tile([P, NB, H], f32, tag="ones_src")
            nc.vector.memset(ones_src, 1.0)
            nc.scalar.copy(
                v_pad.rearrange("p s (h w) -> p s h w", w=DH + 1)[:, :, :, DH],
                ones_src)

            # ---------------- Phase A+B: x transpose + projections ----------
            with tc.tile_pool(name="ab", bufs=1) as Pab, \
                 tc.tile_pool(name="abp", bufs=1, space="PSUM") as PSab:
                xT = Pab.tile([P, NB, S], f32r, tag="xT")
                for sb in range(NB):
                    xin = Pab.tile([P, D], f32, tag="xin", bufs=3)
                    nc.sync.dma_start(out=xin, in_=x_h[sb * P:(sb + 1) * P, :])
                    for g in range(2):
                        ps_tp = PSab.tile([P, 512], f32, tag="tp", bufs=2)
                        for k in range(4):
                            db = 4 * g + k
                            nc.tensor.matmul(ps_tp[:, k * P:(k + 1) * P],
                                             xin[:, db * P:(db + 1) * P], ident,
                                             start=True, stop=True)
                        nc.vector.tensor_copy(
                            xT[:, 4 * g:4 * g + 4, sb * P:(sb + 1) * P],
                            ps_tp.rearrange("p (b w) -> p b w", w=P))

                def load_w(handle):
                    wt = Pab.tile([P, NB, D], f32r, tag="W", bufs=2)
                    for kb in range(NB):
                        nc.gpsimd.dma_start(out=wt[:, kb, :],
                                            in_=handle[kb * P:(kb + 1) * P, :])
                    return wt

                # q projection -> qT (feature-major), scaled by dh^-0.5, +bq
                wq = load_w(wqT_h)
                for m in range(NB):
                    for c in range(2):
                        ps = PSab.tile([P, 512], f32, tag="proj", bufs=4)
                        for kb in range(NB):
                            nc.tensor.matmul(ps, wq[:, kb, m * P:(m + 1) * P],
                                             xT[:, kb, c * 512:(c + 1) * 512],
                                             start=(kb == 0), stop=(kb == NB - 1))
                        nc.scalar.activation(qT[:, m, c * 512:(c + 1) * 512], ps,
                                             Ident, bias=bq_s[:, m:m + 1],
                                             scale=SCALE2)
                # k projection -> kT (feature-major), no bias
                wk = load_w(wkT_h)
                for m in range(NB):
                    for c in range(2):
                        ps = PSab.tile([P, 512], f32, tag="proj", bufs=4)
                        for kb in range(NB):
                            nc.tensor.matmul(ps, wk[:, kb, m * P:(m + 1) * P],
                                             xT[:, kb, c * 512:(c + 1) * 512],
                                             start=(kb == 0), stop=(kb == NB - 1))
                        nc.scalar.copy(kT[:, m, c * 512:(c + 1) * 512], ps)
                # v projection -> v_pad (token-major, bf16, no bias: folded later)
                wv = load_w(wvT_h)
                for st in range(NB):
                    for c in range(2):
                        ps = PSab.tile([P, 512], f32, tag="proj", bufs=4)
                        for kb in range(NB):
                            nc.tensor.matmul(ps, xT[:, kb, st * P:(st + 1) * P],
                                             wv[:, kb, c * 512:(c + 1) * 512],
                                             start=(kb == 0), stop=(kb == NB - 1))
                        dest = v_pad[:, st, :].rearrange(
                            "p (h w) -> p h w", w=DH + 1)[:, 8 * c:8 * c + 8, 0:DH]
                        nc.scalar.copy(dest, ps.rearrange("p (h w) -> p h w", w=DH))

            # ---------------- Phase C+D pools ------------------------------
            with tc.tile_pool(name="cd", bufs=1) as Pcd:
                attnT = Pcd.tile([P, NB, S], f32r, tag="attnT")

                with tc.tile_pool(name="c", bufs=1) as Pc, \
                     tc.tile_pool(name="cp", bufs=1, space="PSUM") as PSc:
                    # mask general blocks (pass1)
                    if g1:
                        mask_g = Pc.tile([P, len(g1) * P], f32, tag="mask_g")
                        for n, (i, j) in enumerate(g1):
                            nc.sync.dma_start(
                                out=mask_g[:, n * P:(n + 1) * P],
                                in_=mask_h[i * P:(i + 1) * P, j * P:(j + 1) * P])
                    # exp(maskT) general blocks (pass2), bf16 in {0..1}
                    if g2:
                        emask_g = Pc.tile([P, len(g2) * P], f32r, tag="emask_g")
                        for n, (t, j) in enumerate(g2):
                            tmpm = Pc.tile([P, P], f32, tag="tmpm", bufs=1)
                            nc.sync.dma_start(
                                out=tmpm,
                                in_=maskT_h[t * P:(t + 1) * P, j * P:(j + 1) * P])
                            nc.scalar.activation(emask_g[:, n * P:(n + 1) * P],
                                                 tmpm, Exp)
                    # qk staging: pooled tiles for finite parts; the all--inf
                    # tails are written straight from a const tile.
                    zsrc = Pc.tile([P, 512], f32, tag="zsrc")
                    nc.vector.memset(zsrc, 0.0)
                    max_tail = max((S - f) for f in fin_end)
                    if max_tail > 0:
                        ninf = Pc.tile([P, max_tail], f32, tag="ninf")
                        nc.vector.memset(ninf, float("-inf"))

                    # per-chunk contributing ktiles and span-nesting check
                    contrib = {}
                    for c in range(2):
                        contrib[c] = [t for t in range(NB)
                                      if min(qe_l[t], (c + 1) * 512) > max(qs_l[t], c * 512)]
                    def span(t, c):
                        return (max(qs_l[t], c * 512), min(qe_l[t], (c + 1) * 512))
                    nested = {}
                    for c in range(2):
                        tcl = contrib[c]
                        nested[c] = bool(tcl) and all(
                            span(t, c)[0] >= span(tcl[0], c)[0]
                            and span(t, c)[1] <= span(tcl[0], c)[1] for t in tcl)

                    for pb in range(NB):  # head pairs; block pb of qT/kT
                        heads = ((0, 2 * pb), (1, 2 * pb + 1))
                        # ---- pass 1: scores + mask -> qk ----
                        for i in range(NB):
                            fin = fin_end[i]
                            stgs = {}
                            ps1s = {}
                            for c in range(2):
                                a = c * 512
                                b = min(fin, a + 512)
                                if b <= a:
                                    continue
                                for hr, h in heads:
                                    if hr not in stgs:
                                        stgs[hr] = Pc.tile([P, S], f32,
                                                           tag="stg", bufs=6,
                                                           name="stg")
                                    ps1 = PSc.tile([P, 512], f32, tag="p1",
                                                   bufs=3, name="ps1")
                                    ps1s[(hr, c)] = ps1
                                    nc.tensor.matmul(
                                        ps1[:, 0:b - a],
                                        qT[64 * hr:64 * hr + 64, pb,
                                           i * P:(i + 1) * P],
                                        kT[64 * hr:64 * hr + 64, pb, a:b],
                                        start=True, stop=True,
                                        tile_position=(64 * hr, 0))
                                for hr, h in heads:
                                    ps1 = ps1s[(hr, c)]
                                    stg = stgs[hr]
                                    row = p1_cls[i]
                                    j = a // P
                                    jend = b // P
                                    while j < jend:
                                        j2 = j
                                        while (j2 + 1 < jend
                                               and row[j2 + 1] == row[j]):
                                            j2 += 1
                                        ra, rb_ = j * P, (j2 + 1) * P
                                        if row[j] == 2:
                                            ga = g1_slot[(i, j)] * P
                                            gb = (g1_slot[(i, j2)] + 1) * P
                                            nc.vector.tensor_add(
                                                stg[:, ra:rb_],
                                                ps1[:, ra - a:rb_ - a],
                                                mask_g[:, ga:gb])
                                        else:
                                            nc.vector.tensor_copy(
                                                stg[:, ra:rb_],
                                                ps1[:, ra - a:rb_ - a])
                                        j = j2 + 1
                            for hr, h in heads:
                                if fin > 0:
                                    nc.sync.dma_start(
                                        out=qk_h[h, i * P:(i + 1) * P, 0:fin],
                                        in_=stgs[hr][:, 0:fin])
                                if fin < S:
                                    nc.sync.dma_start(
                                        out=qk_h[h, i * P:(i + 1) * P, fin:S],
                                        in_=ninf[:, 0:S - fin])

                        if "p2" in skip:
                            continue
                        # ---- pass 2: scores^T, exp, (x emask), PV ----
                        # chunk-sequential; each chunk normalized as soon as
                        # its PV accumulation (and thus its denominators) done
                        dst_odd = Pc.tile([DH, S], f32r, tag="todd", bufs=2,
                                          name="dst_odd")
                        any_pv = False
                        for c in range(2):
                            tcl = contrib[c]
                            if not tcl:
                                continue
                            any_pv = True
                            pv = {}
                            for hr, h in heads:
                                pv[hr] = PSc.tile([DH + 1, 512], f32,
                                                  tag="pv", bufs=2,
                                                  name=f"pv{hr}")
                            for t in tcl:
                                a, b = span(t, c)
                                wts = {}
                                ps2s = {}
                                for hr, h in heads:
                                    ps2 = PSc.tile([P, 512], f32, tag="p2",
                                                   bufs=3, name="ps2")
                                    ps2s[hr] = ps2
                                    nc.tensor.matmul(
                                        ps2[:, 0:b - a],
                                        kT[64 * hr:64 * hr + 64, pb,
                                           t * P:(t + 1) * P],
                                        qT[64 * hr:64 * hr + 64, pb, a:b],
                                        start=True, stop=True,
                                        tile_position=(64 * hr, 0))
                                for hr, h in heads:
                                    wt = Pc.tile([P, 512], f32r, tag="wt",
                                                 bufs=4, name="wt")
                                    wts[hr] = wt
                                    if not nested[c]:
                                        if a > c * 512:
                                            nc.scalar.copy(
                                                wt[:, 0:a - c * 512],
                                                zsrc[:, 0:a - c * 512])
                                        if b < (c + 1) * 512:
                                            nc.scalar.copy(
                                                wt[:, b - c * 512:512],
                                                zsrc[:, 0:(c + 1) * 512 - b])
                                    nc.scalar.activation(
                                        wt[:, a - c * 512:b - c * 512],
                                        ps2s[hr][:, 0:b - a], Exp)
                                    # multiplicative mask on general blocks
                                    row = p2_cls[t]
                                    j0g = qs_l[t] // P
                                    for j in range(a // P, b // P):
                                        if row[j - j0g] == 2:
                                            ga = g2_slot[(t, j)] * P
                                            wa = j * P - c * 512
                                            nc.vector.tensor_mul(
                                                wt[:, wa:wa + P],
                                                wt[:, wa:wa + P],
                                                emask_g[:, ga:ga + P])
                                for hr, h in heads:
                                    if nested[c]:
                                        ra, rb_ = a - c * 512, b - c * 512
                                    else:
                                        ra, rb_ = 0, 512
                                    nc.tensor.matmul(
                                        pv[hr][:, ra:rb_],
                                        v_pad[:, t, h * (DH + 1):(h + 1) * (DH + 1)],
                                        wts[hr][:, ra:rb_],
                                        start=(t == tcl[0]), stop=(t == tcl[-1]))
                            if "norm" in skip:
                                continue
                            # ---- normalize + bias for this chunk ----
                            for hr, h in heads:
                                # denominator lives at partition DH(=64);
                                # custom-DVE and broadcast ops are only correct
                                # at base partition 0: copy to SBUF, DMA-shift.
                                r0 = Pc.tile([DH + 1, 512], f32, tag="r0",
                                             bufs=2)
                                nc.vector.tensor_copy(r0[DH:DH + 1, :],
                                                      pv[hr][DH:DH + 1, :])
                                dn0 = Pc.tile([1, 512], f32, tag="dn0", bufs=2)
                                nc.sync.dma_start(out=dn0, in_=r0[DH:DH + 1, :])
                                rc0 = Pc.tile([1, 512], f32, tag="rc0", bufs=2)
                                nc.vector.reciprocal_approx_fast(out=rc0,
                                                                 in_=dn0)
                                rB = Pc.tile([DH, 512], f32, tag="rB", bufs=2)
                                nc.gpsimd.partition_broadcast(rB, rc0,
                                                              channels=DH)
                                if hr == 0:
                                    dest = attnT[0:DH, pb,
                                                 c * 512:(c + 1) * 512]
                                else:
                                    dest = dst_odd[:, c * 512:(c + 1) * 512]
                                nc.vector.tensor_mul(dest, pv[hr][0:DH, :], rB)
                                nc.vector.tensor_scalar_add(
                                    dest, dest, bv_hd[:, h:h + 1])
                        if any_pv and "norm" not in skip:
                            nc.sync.dma_start(out=attnT[DH:P, pb, :],
                                              in_=dst_odd)

                # ---------------- Phase D: output projection ----------------
                if "d" not in skip:
                  with tc.tile_pool(name="d", bufs=1) as Pd, \
                       tc.tile_pool(name="dp", bufs=1, space="PSUM") as PSd:
                    woT = Pd.tile([P, NB, D], f32r, tag="WoT")
                    for kb in range(NB):
                        nc.gpsimd.dma_start(out=woT[:, kb, :],
                                            in_=woT_h[kb * P:(kb + 1) * P, :])
                    boB = Pd.tile([P, D], f32, tag="boB")
                    bo_ap = bo_h[:]
                    boB_src = bass.AP(tensor=bo_ap.tensor, offset=bo_ap.offset,
                                      ap=[[0, P]] + list(bo_ap.ap))
                    nc.sync.dma_start(out=boB, in_=boB_src)
                    for st in range(NB):
                        outst = Pd.tile([P, D], f32, tag="outst", bufs=2)
                        for c in range(2):
                            ps = PSd.tile([P, 512], f32, tag="op", bufs=4)
                            for kb in range(NB):
                                nc.tensor.matmul(
                                    ps, attnT[:, kb, st * P:(st + 1) * P],
                                    woT[:, kb, c * 512:(c + 1) * 512],
                                    start=(kb == 0), stop=(kb == NB - 1))
                            nc.vector.tensor_add(outst[:, c * 512:(c + 1) * 512],
                                                 ps, boB[:, c * 512:(c + 1) * 512])
                        nc.sync.dma_start(out=out_h[st * P:(st + 1) * P, :],
                                          in_=outst)

    nc.compile()
    return nc


def _prep_inputs(x, mask, Wq, bq, Wk, Wv, bv, Wo, bo):
    """Host-side layout prep (slicing/transposes only, no arithmetic)."""
    x = np.ascontiguousarray(np.asarray(x, np.float32))
    mask = np.ascontiguousarray(np.asarray(mask, np.float32))
    maskT = np.ascontiguousarray(mask.T)
    WqT = np.ascontiguousarray(np.asarray(Wq, np.float32).T)
    WkT = np.ascontiguousarray(np.asarray(Wk, np.float32).T)
    WvT = np.ascontiguousarray(np.asarray(Wv, np.float32).T)
    WoT = np.ascontiguousarray(np.asarray(Wo, np.float32).T)
    bq = np.ascontiguousarray(np.asarray(bq, np.float32))
    bv = np.ascontiguousarray(np.asarray(bv, np.float32))
    bo = np.ascontiguousarray(np.asarray(bo, np.float32))
    shared = {"WqT": WqT, "WkT": WkT, "WvT": WvT, "WoT": WoT,
              "bq": bq, "bv": bv, "bo": bo, "mask": mask, "maskT": maskT}
    in_maps = [dict(shared, x=np.ascontiguousarray(x[b])) for b in range(NCORES)]
    return in_maps, mask


def make_program(mask):
    key = _classify_mask(np.asarray(mask, np.float32))
    key = key + (os.environ.get("MHA_SKIP", ""),)
    return _build(key)


def kernel(x, mask, Wq, bq, Wk, Wv, bv, Wo, bo):
    in_maps, mask_np = _prep_inputs(x, mask, Wq, bq, Wk, Wv, bv, Wo, bo)
    nc = make_program(mask_np)
    from concourse import bass2jax
    results = bass2jax.run_bass_via_pjrt(nc, in_maps, n_cores=NCORES)
    out = np.stack([r["out"] for r in results])
    qk = np.stack([r["qk"] for r in results])
    return out, qk


# revision 22
# speedup vs baseline: 1.0388x; 1.0388x over previous
"""Multi-head attention (Whisper-style, with additive mask) on 8 trn2 NeuronCores.

Batch-parallel: core b computes batch element b entirely on-device:
  q/k/v projections, scores (+mask) -> qk output, softmax, PV, out projection.

Host does layout only: batch slicing, weight transposes (np.ascontiguousarray(W.T)),
mask transpose, and a structural classification of the mask into 128x128 blocks
(all-zero / all--inf / general) that specializes the generated program. All
arithmetic (projections, scores, mask addition, softmax, output projection)
happens on the NeuronCores.
"""
import sys
import os
import functools

sys.path.insert(0, "/opt/trn_rl_repo")
os.environ.setdefault("MYCRO_LOCAL_CACHE", "1")

import numpy as np

B, S, D, H = 8, 1024, 1024, 16
DH = D // H          # 64
P = 128              # partitions
NB = S // P          # 8 blocks of 128
NCORES = 8
SCALE2 = 0.125       # dh ** -0.5 (exact power of two; applied to q side)

_f32 = None  # set lazily


def _dts():
    import concourse.mybir as mybir
    return mybir.dt.float32, mybir.dt.float32r, mybir.dt.bfloat16


def _classify_mask(mask):
    """Classify 128x128 blocks of mask: 0=all-zero, 1=all--inf, 2=general.

    Returns a hashable structure key driving program generation.
    """
    cls = np.zeros((NB, NB), np.int8)
    for i in range(NB):
        for j in range(NB):
            blk = mask[i * P:(i + 1) * P, j * P:(j + 1) * P]
            if np.all(blk == 0.0):
                cls[i][j] = 0
            elif np.all(np.isneginf(blk)):
                cls[i][j] = 1
            else:
                cls[i][j] = 2

    # pass1 (rows = q blocks): trailing all--inf run handled by preset staging
    fin_end = []
    p1_cls = []  # per i: tuple of classes for j < fin blocks (interior -inf -> general)
    for i in range(NB):
        nz = [j for j in range(NB) if cls[i][j] != 1]
        jmax = max(nz) if nz else -1
        fin_end.append(P * (jmax + 1))
        row = []
        for j in range(jmax + 1):
            c = cls[i][j]
            row.append(2 if c == 1 else int(c))  # interior -inf treated as general
        p1_cls.append(tuple(row))

    # pass2 (rows = k blocks of mask^T): clsT[t][j] = cls[j][t]
    qs_l, qe_l, p2_cls = [], [], []
    for t in range(NB):
        nz = [j for j in range(NB) if cls[j][t] != 1]
        if nz:
            jmin, jmax = min(nz), max(nz)
            qs_l.append(P * jmin)
            qe_l.append(P * (jmax + 1))
            row = []
            for j in range(jmin, jmax + 1):
                c = cls[j][t]
                row.append(2 if c == 1 else int(c))
            p2_cls.append(tuple(row))
        else:
            qs_l.append(0)
            qe_l.append(0)
            p2_cls.append(tuple())

    return (tuple(fin_end), tuple(p1_cls), tuple(qs_l), tuple(qe_l),
            tuple(p2_cls))


@functools.lru_cache(maxsize=8)
def _build(key):
    skip = set(os.environ.get("MHA_SKIP", "").split(","))
    import concourse.bass as bass
    from concourse import bacc
    import concourse.mybir as mybir
    import concourse.tile as tile
    from concourse.masks import make_identity

    f32, f32r, bf16 = _dts()
    Ident = mybir.ActivationFunctionType.Identity
    Exp = mybir.ActivationFunctionType.Exp

    fin_end, p1_cls, qs_l, qe_l, p2_cls = key[:5]

    # pass1 general-block slots (i, j) -> slot index in mask_g
    g1 = []
    for i in range(NB):
        for j, c in enumerate(p1_cls[i]):
            if c == 2:
                g1.append((i, j))
    g1_slot = {ij: n for n, ij in enumerate(g1)}
    # pass2 general-block slots (t, j) -> slot in emask_gT
    g2 = []
    for t in range(NB):
        j0 = qs_l[t] // P
        for jj, c in enumerate(p2_cls[t]):
            if c == 2:
                g2.append((t, j0 + jj))
    g2_slot = {tj: n for n, tj in enumerate(g2)}

    nc = bacc.Bacc(None, target_bir_lowering=False)

    x_h = nc.declare_dram_parameter("x", [S, D], f32, isOutput=False)
    wqT_h = nc.declare_dram_parameter("WqT", [D, D], f32, isOutput=False)
    wkT_h = nc.declare_dram_parameter("WkT", [D, D], f32, isOutput=False)
    wvT_h = nc.declare_dram_parameter("WvT", [D, D], f32, isOutput=False)
    woT_h = nc.declare_dram_parameter("WoT", [D, D], f32, isOutput=False)
    bq_h = nc.declare_dram_parameter("bq", [D], f32, isOutput=False)
    bv_h_d = nc.declare_dram_parameter("bv", [D], f32, isOutput=False)
    bo_h = nc.declare_dram_parameter("bo", [D], f32, isOutput=False)
    mask_h = nc.declare_dram_parameter("mask", [S, S], f32, isOutput=False)
    maskT_h = nc.declare_dram_parameter("maskT", [S, S], f32, isOutput=False)
    out_h = nc.declare_dram_parameter("out", [S, D], f32, isOutput=True)
    qk_h = nc.declare_dram_parameter("qk", [H, S, S], f32, isOutput=True)

    with tile.TileContext(nc) as tc:
        with tc.tile_pool(name="persist", bufs=1) as Pp:
            ident = Pp.tile([P, P], f32, tag="ident")
            make_identity(nc, ident)
            # biases
            bq_raw = Pp.tile([P, NB], f32, tag="bq_raw")
            nc.sync.dma_start(out=bq_raw, in_=bq_h[:].rearrange("(b p) -> p b", p=P))
            bq_s = Pp.tile([P, NB], f32, tag="bq_s")
            nc.scalar.activation(bq_s, bq_raw, mybir.ActivationFunctionType.Copy,
                                 scale=SCALE2)
            bv_hd = Pp.tile([DH, H], f32, tag="bv_hd")
            nc.sync.dma_start(out=bv_hd, in_=bv_h_d[:].rearrange("(h j) -> j h", j=DH))

            qT = Pp.tile([P, NB, S], f32r, tag="qT")
            kT = Pp.tile([P, NB, S], f32r, tag="kT")
            v_pad = Pp.tile([P, NB, H * (DH + 1)], f32r, tag="v_pad")
            # ones columns for the softmax denominators (f32 memset + ACT copy
            # so the f32r tile has a rounding producer)
            ones_src = Pp.tile([P, NB, H], f32, tag="ones_src")
            nc.vector.memset(ones_src, 1.0)
            nc.scalar.copy(
                v_pad.rearrange("p s (h w) -> p s h w", w=DH + 1)[:, :, :, DH],
                ones_src)

            # ---------------- Phase A+B: x transpose + projections ----------
            with tc.tile_pool(name="ab", bufs=1) as Pab, \
                 tc.tile_pool(name="abp", bufs=1, space="PSUM") as PSab:
                xT = Pab.tile([P, NB, S], f32r, tag="xT")
                for sb in range(NB):
                    xin = Pab.tile([P, D], f32, tag="xin", bufs=3)
                    nc.sync.dma_start(out=xin, in_=x_h[sb * P:(sb + 1) * P, :])
                    for g in range(2):
                        ps_tp = PSab.tile([P, 512], f32, tag="tp", bufs=2)
                        for k in range(4):
                            db = 4 * g + k
                            nc.tensor.matmul(ps_tp[:, k * P:(k + 1) * P],
                                             xin[:, db * P:(db + 1) * P], ident,
                                             start=True, stop=True)
                        nc.vector.tensor_copy(
                            xT[:, 4 * g:4 * g + 4, sb * P:(sb + 1) * P],
                            ps_tp.rearrange("p (b w) -> p b w", w=P))

                def load_w(handle):
                    wt = Pab.tile([P, NB, D], f32r, tag="W", bufs=2)
                    for kb in range(NB):
                        nc.gpsimd.dma_start(out=wt[:, kb, :],
                                            in_=handle[kb * P:(kb + 1) * P, :])
                    return wt

                # q projection -> qT (feature-major), scaled by dh^-0.5, +bq
                wq = load_w(wqT_h)
                for m in range(NB):
                    for c in range(2):
                        ps = PSab.tile([P, 512], f32, tag="proj", bufs=4)
                        for kb in range(NB):
                            nc.tensor.matmul(ps, wq[:, kb, m * P:(m + 1) * P],
                                             xT[:, kb, c * 512:(c + 1) * 512],
                                             start=(kb == 0), stop=(kb == NB - 1))
                        nc.scalar.activation(qT[:, m, c * 512:(c + 1) * 512], ps,
                                             Ident, bias=bq_s[:, m:m + 1],
                                             scale=SCALE2)
                # k projection -> kT (feature-major), no bias
                wk = load_w(wkT_h)
                for m in range(NB):
                    for c in range(2):
                        ps = PSab.tile([P, 512], f32, tag="proj", bufs=4)
                        for kb in range(NB):
                            nc.tensor.matmul(ps, wk[:, kb, m * P:(m + 1) * P],
                                             xT[:, kb, c * 512:(c + 1) * 512],
                                             start=(kb == 0), stop=(kb == NB - 1))
                        nc.scalar.copy(kT[:, m, c * 512:(c + 1) * 512], ps)
                # v projection -> v_pad (token-major, bf16, no bias: folded later)
                wv = load_w(wvT_h)
                for st in range(NB):
                    for c in range(2):
                        ps = PSab.tile([P, 512], f32, tag="proj", bufs=4)
                        for kb in range(NB):
                            nc.tensor.matmul(ps, xT[:, kb, st * P:(st + 1) * P],
                                             wv[:, kb, c * 512:(c + 1) * 512],
                                             start=(kb == 0), stop=(kb == NB - 1))
                        dest = v_pad[:, st, :].rearrange(
                            "p (h w) -> p h w", w=DH + 1)[:, 8 * c:8 * c + 8, 0:DH]
                        nc.scalar.copy(dest, ps.rearrange("p (h w) -> p h w", w=DH))

            # ---------------- Phase C+D pools ------------------------------
            with tc.tile_pool(name="cd", bufs=1) as Pcd:
                attnT = Pcd.tile([P, NB, S], f32r, tag="attnT")

                with tc.tile_pool(name="c", bufs=1) as Pc, \
                     tc.tile_pool(name="cp", bufs=1, space="PSUM") as PSc:
                    # mask general blocks (pass1)
                    if g1:
                        mask_g = Pc.tile([P, len(g1) * P], f32, tag="mask_g")
                        for n, (i, j) in enumerate(g1):
                            nc.sync.dma_start(
                                out=mask_g[:, n * P:(n + 1) * P],
                                in_=mask_h[i * P:(i + 1) * P, j * P:(j + 1) * P])
                    # exp(maskT) general blocks (pass2), bf16 in {0..1}
                    if g2:
                        emask_g = Pc.tile([P, len(g2) * P], f32r, tag="emask_g")
                        for n, (t, j) in enumerate(g2):
                            tmpm = Pc.tile([P, P], f32, tag="tmpm", bufs=1)
                            nc.sync.dma_start(
                                out=tmpm,
                                in_=maskT_h[t * P:(t + 1) * P, j * P:(j + 1) * P])
                            nc.scalar.activation(emask_g[:, n * P:(n + 1) * P],
                                                 tmpm, Exp)
                    # qk staging: pooled tiles for finite parts; the all--inf
                    # tails are written straight from a const tile.
                    zsrc = Pc.tile([P, 512], f32, tag="zsrc")
                    nc.vector.memset(zsrc, 0.0)
                    max_tail = max((S - f) for f in fin_end)
                    if max_tail > 0:
                        ninf = Pc.tile([P, max_tail], f32, tag="ninf")
                        nc.vector.memset(ninf, float("-inf"))

                    # per-chunk contributing ktiles and span-nesting check
                    contrib = {}
                    for c in range(2):
                        contrib[c] = [t for t in range(NB)
                                      if min(qe_l[t], (c + 1) * 512) > max(qs_l[t], c * 512)]
                    def span(t, c):
                        return (max(qs_l[t], c * 512), min(qe_l[t], (c + 1) * 512))
                    nested = {}
                    for c in range(2):
                        tcl = contrib[c]
                        nested[c] = bool(tcl) and all(
                            span(t, c)[0] >= span(tcl[0], c)[0]
                            and span(t, c)[1] <= span(tcl[0], c)[1] for t in tcl)

                    for pb in range(NB):  # head pairs; block pb of qT/kT
                        heads = ((0, 2 * pb), (1, 2 * pb + 1))
                        # ---- pass 1: scores + mask -> qk ----
                        for i in range(NB):
                            fin = fin_end[i]
                            stgs = {}
                            ps1s = {}
                            for c in range(2):
                                a = c * 512
                                b = min(fin, a + 512)
                                if b <= a:
                                    continue
                                for hr, h in heads:
                                    if hr not in stgs:
                                        stgs[hr] = Pc.tile([P, S], f32,
                                                           tag="stg", bufs=6,
                                                           name="stg")
                                    ps1 = PSc.tile([P, 512], f32, tag="p1",
                                                   bufs=3, name="ps1")
                                    ps1s[(hr, c)] = ps1
                                    nc.tensor.matmul(
                                        ps1[:, 0:b - a],
                                        qT[64 * hr:64 * hr + 64, pb,
                                           i * P:(i + 1) * P],
                                        kT[64 * hr:64 * hr + 64, pb, a:b],
                                        start=True, stop=True,
                                        tile_position=(64 * hr, 0))
                                for hr, h in heads:
                                    ps1 = ps1s[(hr, c)]
                                    stg = stgs[hr]
                                    row = p1_cls[i]
                                    j = a // P
                                    jend = b // P
                                    while j < jend:
                                        j2 = j
                                        while (j2 + 1 < jend
                                               and row[j2 + 1] == row[j]):
                                            j2 += 1
                                        ra, rb_ = j * P, (j2 + 1) * P
                                        if row[j] == 2:
                                            ga = g1_slot[(i, j)] * P
                                            gb = (g1_slot[(i, j2)] + 1) * P
                                            nc.vector.tensor_add(
                                                stg[:, ra:rb_],
                                                ps1[:, ra - a:rb_ - a],
                                                mask_g[:, ga:gb])
                                        else:
                                            nc.vector.tensor_copy(
                                                stg[:, ra:rb_],
                                                ps1[:, ra - a:rb_ - a])
                                        j = j2 + 1
                            for hr, h in heads:
                                if fin > 0:
                                    nc.sync.dma_start(
                                        out=qk_h[h, i * P:(i + 1) * P, 0:fin],
                                        in_=stgs[hr][:, 0:fin])
                                if fin < S:
                                    nc.sync.dma_start(
                                        out=qk_h[h, i * P:(i + 1) * P, fin:S],
                                        in_=ninf[:, 0:S - fin])

                        if "p2" in skip:
                            continue
                        # ---- pass 2: scores^T, exp, (x emask), PV ----
                        # chunk-sequential; each chunk normalized as soon as
                        # its PV accumulation (and thus its denominators) done
                        dst_odd = Pc.tile([DH, S], f32r, tag="todd", bufs=2,
                                          name="dst_odd")
                        any_pv = False
                        for c in range(2):
                            tcl = contrib[c]
                            if not tcl:
                                continue
                            any_pv = True
                            pv = {}
                            for hr, h in heads:
                                pv[hr] = PSc.tile([DH + 1, 512], f32,
                                                  tag="pv", bufs=2,
                                                  name=f"pv{hr}")
                            def issue_scores_exp(t):
                                a, b = span(t, c)
                                wts = {}
                                ps2s = {}
                                for hr, h in heads:
                                    ps2 = PSc.tile([P, 512], f32, tag="p2",
                                                   bufs=3, name="ps2")
                                    ps2s[hr] = ps2
                                    nc.tensor.matmul(
                                        ps2[:, 0:b - a],
                                        kT[64 * hr:64 * hr + 64, pb,
                                           t * P:(t + 1) * P],
                                        qT[64 * hr:64 * hr + 64, pb, a:b],
                                        start=True, stop=True,
                                        tile_position=(64 * hr, 0))
                                for hr, h in heads:
                                    wt = Pc.tile([P, 512], f32r, tag="wt",
                                                 bufs=6, name="wt")
                                    wts[hr] = wt
                                    if not nested[c]:
                                        if a > c * 512:
                                            nc.scalar.copy(
                                                wt[:, 0:a - c * 512],
                                                zsrc[:, 0:a - c * 512])
                                        if b < (c + 1) * 512:
                                            nc.scalar.copy(
                                                wt[:, b - c * 512:512],
                                                zsrc[:, 0:(c + 1) * 512 - b])
                                    nc.scalar.activation(
                                        wt[:, a - c * 512:b - c * 512],
                                        ps2s[hr][:, 0:b - a], Exp)
                                    # multiplicative mask on general blocks
                                    row = p2_cls[t]
                                    j0g = qs_l[t] // P
                                    for j in range(a // P, b // P):
                                        if row[j - j0g] == 2:
                                            ga = g2_slot[(t, j)] * P
                                            wa = j * P - c * 512
                                            nc.vector.tensor_mul(
                                                wt[:, wa:wa + P],
                                                wt[:, wa:wa + P],
                                                emask_g[:, ga:ga + P])
                                return wts

                            def issue_pv(t, wts):
                                a, b = span(t, c)
                                for hr, h in heads:
                                    if nested[c]:
                                        ra, rb_ = a - c * 512, b - c * 512
                                    else:
                                        ra, rb_ = 0, 512
                                    nc.tensor.matmul(
                                        pv[hr][:, ra:rb_],
                                        v_pad[:, t, h * (DH + 1):(h + 1) * (DH + 1)],
                                        wts[hr][:, ra:rb_],
                                        start=(t == tcl[0]), stop=(t == tcl[-1]))

                            prev = None
                            for t in tcl:
                                wts = issue_scores_exp(t)
                                if prev is not None:
                                    issue_pv(*prev)
                                prev = (t, wts)
                            issue_pv(*prev)
                            if "norm" in skip:
                                continue
                            # ---- normalize + bias for this chunk ----
                            for hr, h in heads:
                                # denominator lives at partition DH(=64);
                                # custom-DVE and broadcast ops are only correct
                                # at base partition 0: copy to SBUF, DMA-shift.
                                r0 = Pc.tile([DH + 1, 512], f32, tag="r0",
                                             bufs=2)
                                nc.vector.tensor_copy(r0[DH:DH + 1, :],
                                                      pv[hr][DH:DH + 1, :])
                                dn0 = Pc.tile([1, 512], f32, tag="dn0", bufs=2)
                                nc.sync.dma_start(out=dn0, in_=r0[DH:DH + 1, :])
                                rc0 = Pc.tile([1, 512], f32, tag="rc0", bufs=2)
                                nc.vector.reciprocal_approx_fast(out=rc0,
                                                                 in_=dn0)
                                rB = Pc.tile([DH, 512], f32, tag="rB", bufs=2)
                                nc.gpsimd.partition_broadcast(rB, rc0,
                                                              channels=DH)
                                if hr == 0:
                                    dest = attnT[0:DH, pb,
                                                 c * 512:(c + 1) * 512]
                                else:
                                    dest = dst_odd[:, c * 512:(c + 1) * 512]
                                nc.vector.tensor_mul(dest, pv[hr][0:DH, :], rB)
                                nc.vector.tensor_scalar_add(
                                    dest, dest, bv_hd[:, h:h + 1])
                        if any_pv and "norm" not in skip:
                            nc.sync.dma_start(out=attnT[DH:P, pb, :],
                                              in_=dst_odd)

                # ---------------- Phase D: output projection ----------------
                if "d" not in skip:
                  with tc.tile_pool(name="d", bufs=1) as Pd, \
                       tc.tile_pool(name="dp", bufs=1, space="PSUM") as PSd:
                    woT = Pd.tile([P, NB, D], f32r, tag="WoT")
                    for kb in range(NB):
                        nc.gpsimd.dma_start(out=woT[:, kb, :],
                                            in_=woT_h[kb * P:(kb + 1) * P, :])
                    boB = Pd.tile([P, D], f32, tag="boB")
                    bo_ap = bo_h[:]
                    boB_src = bass.AP(tensor=bo_ap.tensor, offset=bo_ap.offset,
                                      ap=[[0, P]] + list(bo_ap.ap))
                    nc.sync.dma_start(out=boB, in_=boB_src)
                    for st in range(NB):
                        outst = Pd.tile([P, D], f32, tag="outst", bufs=2)
                        for c in range(2):
                            ps = PSd.tile([P, 512], f32, tag="op", bufs=4)
                            for kb in range(NB):
                                nc.tensor.matmul(
                                    ps, attnT[:, kb, st * P:(st + 1) * P],
                                    woT[:, kb, c * 512:(c + 1) * 512],
                                    start=(kb == 0), stop=(kb == NB - 1))
                            nc.vector.tensor_add(outst[:, c * 512:(c + 1) * 512],
                                                 ps, boB[:, c * 512:(c + 1) * 512])
                        nc.sync.dma_start(out=out_h[st * P:(st + 1) * P, :],
                                          in_=outst)

    nc.compile()
    return nc


def _prep_inputs(x, mask, Wq, bq, Wk, Wv, bv, Wo, bo):
    """Host-side layout prep (slicing/transposes only, no arithmetic)."""
    x = np.ascontiguousarray(np.asarray(x, np.float32))
    mask = np.ascontiguousarray(np.asarray(mask, np.float32))
    maskT = np.ascontiguousarray(mask.T)
    WqT = np.ascontiguousarray(np.asarray(Wq, np.float32).T)
    WkT = np.ascontiguousarray(np.asarray(Wk, np.float32).T)
    WvT = np.ascontiguousarray(np.asarray(Wv, np.float32).T)
    WoT = np.ascontiguousarray(np.asarray(Wo, np.float32).T)
    bq = np.ascontiguousarray(np.asarray(bq, np.float32))
    bv = np.ascontiguousarray(np.asarray(bv, np.float32))
    bo = np.ascontiguousarray(np.asarray(bo, np.float32))
    shared = {"WqT": WqT, "WkT": WkT, "WvT": WvT, "WoT": WoT,
              "bq": bq, "bv": bv, "bo": bo, "mask": mask, "maskT": maskT}
    in_maps = [dict(shared, x=np.ascontiguousarray(x[b])) for b in range(NCORES)]
    return in_maps, mask


def make_program(mask):
    key = _classify_mask(np.asarray(mask, np.float32))
    key = key + (os.environ.get("MHA_SKIP", ""),)
    return _build(key)


def kernel(x, mask, Wq, bq, Wk, Wv, bv, Wo, bo):
    in_maps, mask_np = _prep_inputs(x, mask, Wq, bq, Wk, Wv, bv, Wo, bo)
    nc = make_program(mask_np)
    from concourse import bass2jax
    results = bass2jax.run_bass_via_pjrt(nc, in_maps, n_cores=NCORES)
    out = np.stack([r["out"] for r in results])
    qk = np.stack([r["qk"] for r in results])
    return out, qk
